# revision 7
# baseline (speedup 1.0000x reference)
"""Trainium2 Bass kernel: 16-head causal attention with sink logit.

Contract: kernel(**inputs) takes the FULL inputs of the reference
(x [2,2048,1024], W_Q/W_K/W_V/W_out [1024,1024], sink [16]) and returns
the FULL output [2,2048,1024], running on 8 NeuronCores.

Sharding: core c = b*4 + g handles batch b and heads [4g, 4g+4).
Each core computes yT_partial [1024, 2048] = W_out_slice^T @ attn^T;
host sums the 4 partials per batch and transposes.
"""

import sys
import numpy as np

if "/opt/trn_rl_repo" not in sys.path:
    sys.path.insert(0, "/opt/trn_rl_repo")

B, T, C = 2, 2048, 1024
H, D = 16, 64
G = 4                # heads per core
DH = G * D           # 256 head-dims per core
NCORES = 8
QC = 512             # q chunk (matmul moving free dim)
NQ = T // QC         # 4
NKT = T // 128       # 16 k-tiles
NCC = C // 128       # 8 contraction chunks over C
SCALE = 1.0 / float(np.sqrt(D))

# vp_sb per-kt slot layout (386 cols per kt):
#   head0 (even): [V(64) | one]            off 0,   width 65,  denom row 64
#   head1 (odd):  [one | zeros(63) | V(64)] off 65,  width 128, denom row 0
#   head2 (even): [V(64) | one]            off 193, width 65,  denom row 64
#   head3 (odd):  [one | zeros(63) | V(64)] off 258, width 128, denom row 0
VP_W = 386
VP_OFF = [0, 65, 193, 258]
VP_LW = [65, 128, 65, 128]


def build_program(reps=1):
    """Build the per-core Bass program. reps>1 repeats the compute body
    (same inputs -> same outputs) for differential wall-clock timing."""
    from contextlib import ExitStack

    import concourse.bass as bass
    import concourse.tile as tile
    from concourse import bacc, mybir

    f32 = mybir.dt.float32
    f32r = mybir.dt.float32r
    AF = mybir.ActivationFunctionType
    Alu = mybir.AluOpType

    nc = bacc.Bacc("TRN2", target_bir_lowering=False, debug=False)

    xt_d = nc.dram_tensor("xt", [C, T], f32r, kind="ExternalInput").ap()
    wq_d = nc.dram_tensor("wq", [C, DH], f32r, kind="ExternalInput").ap()
    wk_d = nc.dram_tensor("wk", [C, DH], f32r, kind="ExternalInput").ap()
    wv_d = nc.dram_tensor("wv", [C, DH], f32r, kind="ExternalInput").ap()
    wo_d = nc.dram_tensor("wo", [DH, C], f32r, kind="ExternalInput").ap()
    sk_d = nc.dram_tensor("sk", [1, G], f32, kind="ExternalInput").ap()
    cm_d = nc.dram_tensor("cm", [128, 4 * QC], f32, kind="ExternalInput").ap()
    vpc_d = nc.dram_tensor("vpc", [128, 65], f32r, kind="ExternalInput").ap()
    onr_d = nc.dram_tensor("onr", [1, 128], f32r, kind="ExternalInput").ap()
    yt_d = nc.dram_tensor("yt", [C, T], f32, kind="ExternalOutput").ap()

    xt_v = xt_d.rearrange("(n p) m -> p n m", p=128)   # [128, 8, 2048]
    wq_v = wq_d.rearrange("(n p) m -> p n m", p=128)   # [128, 8, 256]
    wk_v = wk_d.rearrange("(n p) m -> p n m", p=128)
    wv_v = wv_d.rearrange("(n p) m -> p n m", p=128)
    wo_v = wo_d.rearrange("(n p) m -> p n m", p=128)   # [128, 2, 1024]
    yt_v = yt_d.rearrange("(n p) m -> p n m", p=128)   # [128, 8, 2048]

    with tile.TileContext(nc) as tc, ExitStack() as ctx:
        P = lambda name, bufs: ctx.enter_context(tc.tile_pool(name=name, bufs=bufs))
        const_p = P("const", 1)
        big_p = P("big", 1)
        p_p = P("p", 4)
        praw_p = P("praw", 2)
        y_p = P("y", 2)
        bcs_p = P("bcs", 2)
        row_p = P("row", 2)
        ps_p = ctx.enter_context(tc.tile_pool(name="ps", bufs=4, space="PSUM"))
        o_p = ctx.enter_context(tc.tile_pool(name="o", bufs=4, space="PSUM"))

        # ---- persistent SBUF tensors ----
        xt_sb = big_p.tile([128, NCC * T], f32r, tag="xt")           # 64KB/part
        wq_sb = big_p.tile([128, NCC * DH], f32r, tag="wq")
        wk_sb = big_p.tile([128, NCC * DH], f32r, tag="wk")
        wv_sb = big_p.tile([128, NCC * DH], f32r, tag="wv")
        wo_sb = big_p.tile([128, 2 * C], f32r, tag="wo")
        qt_sb = big_p.tile([128, 2 * T], f32r, tag="qt")
        kt_sb = big_p.tile([128, 2 * T], f32r, tag="kt")
        vp_sb = big_p.tile([128, NKT * VP_W], f32r, tag="vp")
        at_sb = big_p.tile([128, 2 * T], f32r, tag="at")             # attn^T normalized
        cm_sb = const_p.tile([128, 4 * QC], f32, tag="cm")
        ones_sb = const_p.tile([128, 128], f32r, tag="ones")
        skr_sb = const_p.tile([128, G], f32, tag="skr")
        esk_sb = const_p.tile([128, G], f32, tag="esk")

        # ---- phase 0: loads + constants ----
        for i in range(NCC):
            nc.sync.dma_start(xt_sb[:, i * T:(i + 1) * T], xt_v[:, i, :])
        nc.sync.dma_start(
            wq_sb[:].rearrange("p (n m) -> p n m", m=DH), wq_v[:, :, :])
        nc.sync.dma_start(
            wk_sb[:].rearrange("p (n m) -> p n m", m=DH), wk_v[:, :, :])
        nc.sync.dma_start(
            wv_sb[:].rearrange("p (n m) -> p n m", m=DH), wv_v[:, :, :])
        nc.sync.dma_start(
            wo_sb[:].rearrange("p (n m) -> p n m", m=C), wo_v[:, :, :])
        nc.sync.dma_start(cm_sb[:, :], cm_d[:, :])
        nc.sync.dma_start(skr_sb[0:1, :], sk_d[:, :])
        nc.sync.dma_start(skr_sb[64:65, :], sk_d[:, :])
        nc.scalar.activation(esk_sb[0:1, :], skr_sb[0:1, :], AF.Exp)
        nc.scalar.activation(esk_sb[64:65, :], skr_sb[64:65, :], AF.Exp)
        nc.sync.dma_start(ones_sb[0:1, :], onr_d[:, :])
        nc.sync.dma_start(ones_sb[64:65, :], onr_d[:, :])
        # vp ones columns and zero filler ([1,1,0*63] pattern per region)
        for kt in range(NKT):
            base = kt * VP_W
            nc.sync.dma_start(vp_sb[:, base + 64:base + 129], vpc_d[:, :])
            nc.sync.dma_start(vp_sb[:, base + 257:base + 322], vpc_d[:, :])

        for _ in range(reps):
            # ---- phase 1: Q^T and K^T projections  [d(128/pair), t] ----
            for w_sb, t_sb in ((wq_sb, qt_sb), (wk_sb, kt_sb)):
                for mt in range(2):           # head pair -> 128 d rows
                    for qc in range(NQ):
                        ps = ps_p.tile([128, QC], f32, tag="ps")
                        for ci in range(NCC):
                            nc.tensor.matmul(
                                ps[:, :],
                                w_sb[:, ci * DH + mt * 128: ci * DH + (mt + 1) * 128],
                                xt_sb[:, ci * T + qc * QC: ci * T + qc * QC + QC],
                                start=(ci == 0), stop=(ci == NCC - 1))
                        nc.vector.tensor_copy(
                            t_sb[:, mt * T + qc * QC: mt * T + (qc) * QC + QC], ps[:, :])

            # ---- phase 1b: V natural [t, d] into padded vp layout ----
            for tt in range(NKT):
                ps = ps_p.tile([128, DH], f32, tag="ps")
                for ci in range(NCC):
                    nc.tensor.matmul(
                        ps[:, :],
                        xt_sb[:, ci * T + tt * 128: ci * T + (tt + 1) * 128],
                        wv_sb[:, ci * DH: (ci + 1) * DH],
                        start=(ci == 0), stop=(ci == NCC - 1))
                base = tt * VP_W
                nc.vector.tensor_copy(vp_sb[:, base + 0: base + 64], ps[:, 0:64])
                nc.vector.tensor_copy(vp_sb[:, base + 129: base + 257], ps[:, 64:192])
                nc.vector.tensor_copy(vp_sb[:, base + 322: base + 386], ps[:, 192:256])

            # ---- phase 2: attention per head-pair ----
            for p in range(2):
                hA, hB = 2 * p, 2 * p + 1
                for qc in range(NQ):
                    nkt = 4 * qc + 4
                    oA = o_p.tile([128, QC], f32, tag="o")   # rows 0-63 attn, 64 denom
                    oB = o_p.tile([128, QC], f32, tag="o")   # row 0 denom, 64-127 attn
                    for kt in range(nkt):
                        sA = ps_p.tile([128, QC], f32, tag="ps")
                        sB = ps_p.tile([128, QC], f32, tag="ps")
                        nc.tensor.matmul(
                            sA[:, :],
                            kt_sb[0:64, p * T + kt * 128: p * T + (kt + 1) * 128],
                            qt_sb[0:64, p * T + qc * QC: p * T + qc * QC + QC],
                            start=True, stop=True)
                        nc.tensor.matmul(
                            sB[:, :],
                            kt_sb[64:128, p * T + kt * 128: p * T + (kt + 1) * 128],
                            qt_sb[64:128, p * T + qc * QC: p * T + qc * QC + QC],
                            start=True, stop=True)
                        diag = kt - 4 * qc
                        if diag >= 0:
                            pA = p_p.tile([128, QC], f32r, tag="p")
                            pB = p_p.tile([128, QC], f32r, tag="p")
                            prA = praw_p.tile([128, QC], f32, tag="praw")
                            prB = praw_p.tile([128, QC], f32, tag="praw")
                            nc.scalar.activation(prA[:, :], sA[:, :], AF.Exp, scale=SCALE)
                            nc.scalar.activation(prB[:, :], sB[:, :], AF.Exp, scale=SCALE)
                            msk = cm_sb[:, diag * QC:(diag + 1) * QC]
                            nc.vector.tensor_mul(pA[:, :], prA[:, :], msk)
                            nc.vector.tensor_mul(pB[:, :], prB[:, :], msk)
                        else:
                            pA = p_p.tile([128, QC], f32r, tag="p")
                            pB = p_p.tile([128, QC], f32r, tag="p")
                            nc.scalar.activation(pA[:, :], sA[:, :], AF.Exp, scale=SCALE)
                            nc.scalar.activation(pB[:, :], sB[:, :], AF.Exp, scale=SCALE)
                        base = kt * VP_W
                        nc.tensor.matmul(
                            oA[0:65, :],
                            vp_sb[:, base + VP_OFF[hA]: base + VP_OFF[hA] + 65],
                            pA[:, :],
                            start=(kt == 0), stop=(kt == nkt - 1))
                        nc.tensor.matmul(
                            oB[:, :],
                            vp_sb[:, base + VP_OFF[hB]: base + VP_OFF[hB] + 128],
                            pB[:, :],
                            start=(kt == 0), stop=(kt == nkt - 1))

                    # normalize: head A denom at row 64 of oA, head B denom
                    # at row 0 of oB; shared tiles, disjoint partition ranges.
                    dn = row_p.tile([128, QC], f32, tag="row")
                    rc = row_p.tile([128, QC], f32r, tag="rowr")
                    bc = ps_p.tile([128, QC], f32, tag="ps")
                    bcs = bcs_p.tile([128, QC], f32, tag="bcs")
                    nc.vector.tensor_scalar(
                        out=dn[64:65, :], in0=oA[64:65, :],
                        scalar1=esk_sb[64:65, hA:hA + 1], scalar2=None, op0=Alu.add)
                    nc.vector.tensor_scalar(
                        out=dn[0:1, :], in0=oB[0:1, :],
                        scalar1=esk_sb[0:1, hB:hB + 1], scalar2=None, op0=Alu.add)
                    with nc.allow_low_precision(reason="f32r recip for PE broadcast"):
                        nc.vector.reciprocal(rc[64:65, :], dn[64:65, :])
                        nc.vector.reciprocal(rc[0:1, :], dn[0:1, :])
                    bc2 = ps_p.tile([128, QC], f32, tag="ps")
                    nc.tensor.matmul(
                        bc[:, :], ones_sb[64:65, :], rc[64:65, :],
                        start=True, stop=True)
                    nc.tensor.matmul(
                        bc2[:, :], ones_sb[0:1, :], rc[0:1, :],
                        start=True, stop=True)
                    nc.vector.tensor_copy(bcs[0:64, :], bc[0:64, :])
                    nc.vector.tensor_copy(bcs[64:128, :], bc2[64:128, :])
                    nc.vector.tensor_mul(
                        at_sb[0:64, p * T + qc * QC: p * T + qc * QC + QC],
                        oA[0:64, :], bcs[0:64, :])
                    nc.vector.tensor_mul(
                        at_sb[64:128, p * T + qc * QC: p * T + qc * QC + QC],
                        oB[64:128, :], bcs[64:128, :])

            # ---- phase 3: y^T = W_out_slice^T @ attn^T ----
            for co in range(NCC):
                for qc in range(NQ):
                    ps = ps_p.tile([128, QC], f32, tag="ps")
                    for j in range(2):
                        nc.tensor.matmul(
                            ps[:, :],
                            wo_sb[:, j * C + co * 128: j * C + (co + 1) * 128],
                            at_sb[:, j * T + qc * QC: j * T + qc * QC + QC],
                            start=(j == 0), stop=(j == 1))
                    yt = y_p.tile([128, QC], f32, tag="y")
                    nc.vector.tensor_copy(yt[:, :], ps[:, :])
                    nc.sync.dma_start(yt_v[:, co, qc * QC: qc * QC + QC], yt[:, :])

    nc.compile()
    return nc


def make_causal_masks():
    cm = np.zeros((128, 4 * QC), dtype=np.float32)
    kl = np.arange(128)[:, None]
    ql = np.arange(QC)[None, :]
    for m in range(4):
        cm[:, m * QC:(m + 1) * QC] = (ql >= kl + 128 * m).astype(np.float32)
    return cm


def shard_inputs(x, W_Q, W_K, W_V, W_out, sink):
    cm = make_causal_masks()
    vpc = np.zeros((128, 65), dtype=np.float32)
    vpc[:, 0:2] = 1.0
    in_maps = []
    for c in range(NCORES):
        b, g = divmod(c, G)
        cols = slice(g * DH, (g + 1) * DH)
        in_maps.append({
            "xt": np.ascontiguousarray(x[b].T),
            "wq": np.ascontiguousarray(W_Q[:, cols]),
            "wk": np.ascontiguousarray(W_K[:, cols]),
            "wv": np.ascontiguousarray(W_V[:, cols]),
            "wo": np.ascontiguousarray(W_out[cols, :]),
            "sk": np.ascontiguousarray(sink[g * G:(g + 1) * G][None, :]),
            "cm": cm,
            "vpc": vpc,
            "onr": np.ones((1, 128), dtype=np.float32),
        })
    return in_maps


def gather_outputs(results):
    out = np.zeros((B, T, C), dtype=np.float32)
    for b in range(B):
        acc = np.zeros((C, T), dtype=np.float32)
        for g in range(G):
            acc += results[b * G + g]["yt"]
        out[b] = acc.T
    return out


_CACHE = {}


def _get_program():
    if "nc" not in _CACHE:
        _CACHE["nc"] = build_program(reps=1)
    return _CACHE["nc"]


def kernel(x, W_Q, W_K, W_V, W_out, sink):
    from concourse.bass_utils import run_bass_kernel_spmd

    x = np.asarray(x, dtype=np.float32)
    W_Q = np.asarray(W_Q, dtype=np.float32)
    W_K = np.asarray(W_K, dtype=np.float32)
    W_V = np.asarray(W_V, dtype=np.float32)
    W_out = np.asarray(W_out, dtype=np.float32)
    sink = np.asarray(sink, dtype=np.float32)

    nc = _get_program()
    in_maps = shard_inputs(x, W_Q, W_K, W_V, W_out, sink)
    res = run_bass_kernel_spmd(nc, in_maps, core_ids=list(range(NCORES)))
    return gather_outputs(res.results)
